# revision 21
# baseline (speedup 1.0000x reference)
"""Trainium2 Bass kernel for nn_BaseAttention (B=4, H=16, S=2048, D=64, key-mask).

Strategy (8 NeuronCores, batch*head sharded, 8 heads per core; each core's 8
heads share one batch's mask):

* Host-side packing/layout (index gather + transposes only, no math):
  - The key mask is per-(batch, key) and masks ~half the keys with -1e4,
    whose exp underflows to exactly 0 in f32.  kernel() gathers the unmasked
    keys of K and V per batch and zero-pads to a common capacity cap
    (multiple of 128) — identical math, ~half the exp/matmul work.
  - Q is shipped pre-transposed as [2D, S] (Q^T duplicated into both
    partition halves), K as the pair-stacked transpose [128, T2*128] with
    row c*64+d, col u*128+p = K[(2u+c)*128+p, d].  So the device does no
    transposes at all, and mm1 can run k-tile pairs concurrently in the two
    row halves of the PE array (row tiling).
  - V' = [V | ones | 0-pad] -> [cap, 80]; the ones column gives the softmax
    denominator via mm2 (zero for padded keys).
  - The kernel stores out'^T = [64, S] (numerator/denominator already
    divided); the host transposes back to [S, 64].

* Per head on device:
  - mm1 (k-pair-major): scores S^T[t] = Kp[t] @ Q^T land in a hand-sliced
    6-slot PSUM ring ([128, 6, 512] f32); a k-tile pair (rows 0-63 / 64-127)
    fills 4 slots; one ScalarE ACTIVATE with a (possibly wrapped) 4-slot AP
    computes P^T = Exp(S^T/8) with N=2048, amortizing the ~350-cycle
    ACTIVATE overhead.  No max-subtraction: scores ~N(0,1) after the 1/8
    scale; padded keys give exp(0)=1 but their V' rows are all-zero.
  - mm2 window-major: acc[w] [80,512] f32 accumulates V'[t]^T P^T[t] over
    all t; 2 acc banks (PSUM = 6 + 2 = 8 banks).  P^T pair-chunks stay
    parked in SBUF.
  - Per window: drain acc [80,512] to SBUF f32.  Per head: broadcast the
    sums row across partitions (GpSimd) and divide (DVE), store out'^T.
* Emission is a flat software pipeline over (head, pair, q-half) chunks;
  head h's mm2/epilogue work is spread across head h+1's chunks.

Self-contained: hardcodes shapes; imports concourse from /opt/trn_rl_repo.
"""

import sys

if "/opt/trn_rl_repo" not in sys.path:
    sys.path.insert(0, "/opt/trn_rl_repo")

import numpy as np

import concourse.bass as bass
import concourse.mybir as mybir
import concourse.tile as tile
from concourse import bacc

F32 = mybir.dt.float32
BF16 = mybir.dt.bfloat16

N_CORES = 8
B, NH, S, D = 4, 16, 2048, 64
H = (B * NH) // N_CORES  # heads per core = 8
P = 128                  # partitions / k-tile size
W = 512                  # q window width (PSUM fp32 bank)
NW = S // W              # 4 windows
VC = 128                 # V' columns: 64 v + 1 ones + 63 zero pad (FWL wants 128)
NSLOT = 6                # score ring slots ([128, 512] f32, 1 bank each)
SCALE = 1.0 / 8.0        # 1/sqrt(D)


def emit_core_program(ctx, nc, tc, T, q_h, k_h, v_h, out_h):
    """Per-core program.

    q: [H, 2D, S] (Q^T, both halves); k: [H, 128, T2*128] (pair-stacked K^T);
    v: [H, T*128, 80]; out: [H, D, S] (= out'^T).
    """
    T2 = (T + 1) // 2
    pool = lambda *a, **kw: ctx.enter_context(tc.tile_pool(*a, **kw))
    ld = pool(name="ld", bufs=3)              # qT/kT/V' staging (bf16)
    ppool = pool(name="p", bufs=T2 + 3)       # P^T pair chunks [128, 2, 2048]
    accs_pool = pool(name="accs", bufs=2)     # drained accumulators (f32)
    rep_pool = pool(name="rep", bufs=2)       # broadcast denominators
    ost_pool = pool(name="ost", bufs=2)       # output staging f32

    st_pool = pool(name="stp", bufs=1, space="PSUM")
    st = st_pool.tile([P, NSLOT, W], F32, name="st")      # 6-bank score ring
    acc_pool = pool(name="acc", bufs=1, space="PSUM")     # 2 banks

    def emit_head_load(h):
        qT = ld.tile([P, S], BF16, tag="qT", name=f"qT_{h}")
        nc.gpsimd.dma_start(out=qT, in_=q_h[h])
        kT = ld.tile([P, T2 * P], BF16, tag="kT", name=f"kT_{h}")
        nc.gpsimd.dma_start(out=kT, in_=k_h[h])
        v_sb = ld.tile([P, T, VC], BF16, tag="v_sb", name=f"v_sb_{h}")
        nc.gpsimd.dma_start(out=v_sb, in_=v_h[h].rearrange("(t p) c -> p t c", p=P))
        return qT, kT, v_sb

    slot_ctr = [0]

    def st_ap(s0, nslots):
        # AP over ring slots s0..s0+nslots-1 (mod NSLOT), as [128, n/2, 2, W]
        base = st[:, 0, :]
        pdim = st.ap[0]
        ap = [pdim]
        if nslots == 4:
            delta = (s0 + 2) % NSLOT - s0
            ap.append([delta * W, 2])
        ap += [[W, 2], [1, W]]
        return bass.AP(tensor=st.tensor, offset=st.offset + s0 * W, ap=ap)

    def emit_chunk(h, u, c2):
        # one q-half of k-tile pair u: 2 matmuls + one N=1024 ACTIVATE per
        # member (2-slot chunks in the 6-slot ring = 3 chunks of lookahead,
        # so the ScalarE never waits on an mm1 tail)
        qT, kT, _ = heads[h]
        members = [c for c in range(2) if 2 * u + c < T]
        s0 = {}
        for m in members:
            s0[m] = slot_ctr[0] % NSLOT
            slot_ctr[0] += 2
        for c in range(2):  # interleave halves for row-tiling concurrency
            for m in members:
                lo = m * D
                nc.tensor.matmul(
                    st[:, (s0[m] + c) % NSLOT, :],
                    lhsT=kT[lo : lo + D, u * P : (u + 1) * P],
                    rhs=qT[lo : lo + D, c2 * 1024 + c * W : c2 * 1024 + (c + 1) * W],
                    start=True,
                    stop=True,
                )
        if c2 == 0:
            pTs[(h, u)] = ppool.tile([P, 2, S], BF16, tag="pT", name=f"pT_{h}_{u}")
        pT = pTs[(h, u)]
        pr = pT.rearrange("p a (b w) -> p a b w", w=W)
        for m in members:
            nc.scalar.activation(
                out=pr[:, m, 2 * c2 : 2 * c2 + 2, :],
                in_=st_ap(s0[m], 2),
                func=mybir.ActivationFunctionType.Exp,
                scale=SCALE,
            )

    OW = 256  # mm2 quad window width (4 x 256 f32 = 2 PSUM banks)

    def mm2_piece_thunk(h, p, t):
        # one weight-load of V'[t] serves 4 N=256 matmuls (q half p)
        def f():
            v_sb = heads[h][2]
            if t == 0:
                acc = acc_pool.tile([VC, 4, OW], F32, tag="acc", name=f"acc_{h}_{p}")
                accs_by_hw[(h, p)] = acc
            acc = accs_by_hw[(h, p)]
            for o in range(4):
                # start=True clears has_written for the WHOLE bank, so only
                # the first octet-group of each bank may set it; the other
                # group's first write lands on cleared bits (= overwrite).
                nc.tensor.matmul(
                    acc[:, o, :],
                    lhsT=v_sb[:, t, :],
                    rhs=pTs[(h, t // 2)][
                        :, t % 2, p * 1024 + o * OW : p * 1024 + (o + 1) * OW
                    ],
                    start=(t == 0 and o % 2 == 0),
                    stop=(t == T - 1),
                    skip_group_check=True,
                )
            if p == 1 and t == T - 1:
                for u in range((T + 1) // 2):
                    del pTs[(h, u)]
        return f

    def drain_thunk(h, p):
        def f():
            acc = accs_by_hw.pop((h, p))
            if p == 0:
                accs_by_head[h] = accs_pool.tile(
                    [VC, 2, 4, OW], F32, tag="accs", name=f"accs_{h}"
                )
            nc.vector.tensor_copy(accs_by_head[h][:, p, :, :], acc)
        return f

    def epilogue_thunk(h):
        def f():
            accs = accs_by_head.pop(h)
            sums = rep_pool.tile([1, S], F32, tag="sums")
            nc.vector.tensor_copy(
                sums, accs[D : D + 1, :, :, :].rearrange("c p o w -> c (p o w)")
            )
            rec = rep_pool.tile([1, S], F32, tag="rec")
            nc.vector.reciprocal_approx_fast(rec, sums)
            rep = rep_pool.tile([D, S], F32, tag="rep")
            nc.gpsimd.partition_broadcast(rep, rec, channels=D)
            ost = ost_pool.tile([D, S], F32, tag="ost")
            nc.vector.tensor_mul(
                ost, accs.rearrange("c p o w -> c (p o w)")[0:D, :], rep
            )
            nc.sync.dma_start(out=out_h[h], in_=ost)
        return f

    units = [(h, u, c2) for h in range(H) for u in range(T2) for c2 in range(2)]
    heads = {0: emit_head_load(0)}
    pTs = {}
    accs_by_hw = {}
    accs_by_head = {}
    work = []

    for i, (h, u, c2) in enumerate(units):
        if u == 0 and c2 == 0 and h + 1 < H:
            heads[h + 1] = emit_head_load(h + 1)
        if u == 1 and c2 == 0 and h >= 2:
            del heads[h - 2]
        emit_chunk(h, u, c2)
        if u == T2 - 1 and c2 == 1:  # head's chunks all emitted: queue work
            for p in range(2):
                for t in range(T):
                    work.append(mm2_piece_thunk(h, p, t))
                work.append(drain_thunk(h, p))
            work.append(epilogue_thunk(h))
        # spread queued work across the next head's chunks
        nthunks = 2 * T + 3
        npop = -(-nthunks // (2 * T2))
        for _ in range(npop):
            if work:
                work.pop(0)()
    while work:
        work.pop(0)()


def build_nc(T):
    T2 = (T + 1) // 2
    nc = bacc.Bacc("TRN2", target_bir_lowering=False, debug=False, num_devices=N_CORES)
    q = nc.declare_dram_parameter("q", [H, 2 * D, S], F32, isOutput=False)
    k = nc.declare_dram_parameter("k", [H, P, T2 * P], F32, isOutput=False)
    v = nc.declare_dram_parameter("v", [H, T * P, VC], F32, isOutput=False)
    out = nc.declare_dram_parameter("out", [H, D, S], F32, isOutput=True)
    from contextlib import ExitStack

    with tile.TileContext(nc) as tc, ExitStack() as ctx:
        emit_core_program(ctx, nc, tc, T, q.ap(), k.ap(), v.ap(), out.ap())
    nc.compile()
    return nc


_NC_CACHE = {}


def get_nc(T):
    if T not in _NC_CACHE:
        _NC_CACHE[T] = build_nc(T)
    return _NC_CACHE[T]


def make_in_maps(q, k, v, mask):
    """Pack unmasked keys per batch; build device layouts; shard 8 cores."""
    qf = np.asarray(q, dtype=np.float32)
    kf = np.asarray(k, dtype=np.float32)
    vf = np.asarray(v, dtype=np.float32)
    mf = np.asarray(mask, dtype=np.int32).reshape(B, S)

    idxs = [np.flatnonzero(mf[b] == 0) for b in range(B)]
    maxcnt = max(len(ix) for ix in idxs)
    cap = min(S, max(P, -(-maxcnt // P) * P))
    T = cap // P
    T2 = (T + 1) // 2

    kp = np.zeros((B, NH, T2 * 2 * P, D), dtype=np.float32)
    vp = np.zeros((B, NH, cap, VC), dtype=np.float32)
    for b in range(B):
        n = len(idxs[b])
        kp[b, :, :n, :] = kf[b][:, idxs[b], :]
        vp[b, :, :n, :D] = vf[b][:, idxs[b], :]
        vp[b, :, :n, D] = 1.0

    # Q^T duplicated into both partition halves: [B,NH,S,D] -> [BNH, 2D, S]
    qT = qf.reshape(B * NH, S, D).transpose(0, 2, 1)
    qTd = np.concatenate([qT, qT], axis=1)
    # pair-stacked K^T: row c*64+d, col u*128+p = K[(2u+c)*128+p, d]
    k4 = kp.reshape(B * NH, T2, 2, P, D).transpose(0, 2, 4, 1, 3)
    kTd = k4.reshape(B * NH, 2 * D, T2 * P)
    vp = vp.reshape(B * NH, cap, VC)

    in_maps = []
    for c in range(N_CORES):
        lo = c * H
        in_maps.append(
            {
                "q": np.ascontiguousarray(qTd[lo : lo + H]),
                "k": np.ascontiguousarray(kTd[lo : lo + H]),
                "v": np.ascontiguousarray(vp[lo : lo + H]),
            }
        )
    return T, in_maps


def kernel(q, k, v, mask):
    from concourse.bass_utils import run_bass_kernel_spmd

    T, in_maps = make_in_maps(q, k, v, mask)
    nc = get_nc(T)
    try:
        res = run_bass_kernel_spmd(nc, in_maps, list(range(N_CORES))).results
    except Exception:
        # the axon execute path occasionally throws a transient INTERNAL
        # error right after a fresh NEFF compile; one retry clears it
        res = run_bass_kernel_spmd(nc, in_maps, list(range(N_CORES))).results
    # out is [H, D, S] per core (= out'^T): gather + host de-transpose
    out = np.concatenate([res[c]["out"] for c in range(N_CORES)], axis=0)
    return np.ascontiguousarray(out.transpose(0, 2, 1)).reshape(B, NH, S, D)


if __name__ == "__main__":
    nc = build_nc(int(sys.argv[1]) if len(sys.argv) > 1 else 9)
    print("built ok")


# revision 26
# speedup vs baseline: 1.1595x; 1.1595x over previous
"""Trainium2 Bass kernel for nn_BaseAttention (B=4, H=16, S=2048, D=64, key-mask).

Strategy (8 NeuronCores, batch*head sharded, 8 heads per core; each core's 8
heads share one batch's mask):
  Host-side key packing: the key mask is per-(batch, key) and masks ~half the
  keys with -1e4 (whose exp underflows to exactly 0 in f32).  kernel() gathers
  the unmasked keys of K and V per batch, appends a "ones" column to V (for
  the softmax denominator), and zero-pads to a common capacity `cap` (multiple
  of 256).  The device kernel then runs dense attention over cap keys instead
  of 2048 — exactly equivalent math, ~half the exp/matmul work.

  Per head on device (Q: [S,D], Kp: [cap,D], Vp': [cap,D+1], all f32 in HBM):
    - Load with fp32->bf16 cast during DMA (SWDGE).
    - PE-transpose Q,Kp tiles -> Q^T [64,S], Kp^T [64,cap] bf16, duplicated
      onto partitions 64-127 so mm1 can run two k-tiles in the two row halves.
    - Scores transposed: S^T[k, q] = Kp @ Q^T, fp32 PSUM, one [128, 2*512]
      tile per k-tile pair; one ScalarE pass computes P^T = Exp(S^T/8).
      No max-subtraction (scores ~N(0,1)); no additive mask (handled by the
      packing; padded K rows give exp(0)=1 but their V' rows are zero).
    - mm2 accumulates out'^T [65, q] over k; the ones-column row is the
      softmax denominator (zero for padded keys).
    - Reciprocal of sums, PE-transpose [65, q] -> [q, 65], scale, store.
  Emission is a flat software pipeline over (head, window, k-pair) units with
  mm2 and epilogues lagging so the in-order PE stream never stalls.

Self-contained: hardcodes shapes; imports concourse from /opt/trn_rl_repo.
"""

import sys

if "/opt/trn_rl_repo" not in sys.path:
    sys.path.insert(0, "/opt/trn_rl_repo")

import numpy as np

import concourse.bass as bass
import concourse.mybir as mybir
import concourse.tile as tile
from concourse import bacc
from concourse.masks import make_identity

F32 = mybir.dt.float32
BF16 = mybir.dt.bfloat16
I32 = mybir.dt.int32

N_CORES = 8
B, NH, S, D = 4, 16, 2048, 64
H = (B * NH) // N_CORES  # heads per core = 8
P = 128                  # partitions / k-tile size
W = 512                  # q-window width (= fp32 PSUM bank limit per matmul)
NW = S // W              # 4 q-windows per head
SCALE = 1.0 / 8.0        # 1/sqrt(D)


def emit_core_program(ctx, nc, tc, T, q_h, k_h, v_h, out_h):
    """Per-core Tile program. q/out: [H, S, D]; k: [H, T*128, D]; v: [H, T*128, D+1]."""
    cap = T * P
    pool = lambda *a, **kw: ctx.enter_context(tc.tile_pool(*a, **kw))
    singles = pool(name="singles", bufs=1)
    ld = pool(name="ld", bufs=2)            # SBUF head staging (bf16)
    qkT = pool(name="qkT", bufs=2)          # SBUF Q^T/K^T (both row halves)
    ppool = pool(name="p", bufs=5)          # SBUF P^T tiles (lagged mm2)
    accs_pool = pool(name="accs", bufs=2)   # SBUF drained accumulators
    outs_pool = pool(name="outs", bufs=2)   # SBUF output staging
    st_pool = pool(name="st", bufs=2, space="PSUM")    # S^T pair tiles (2 banks ea)
    acc_pool = pool(name="acc", bufs=2, space="PSUM")  # out'^T accum (1 bank ea)
    tp_pool = pool(name="tp", bufs=2, space="PSUM")    # transposes (1 bank ea)

    ident_bf = singles.tile([P, P], BF16)
    make_identity(nc, ident_bf)
    ident_f32 = singles.tile([P, P], F32)
    make_identity(nc, ident_f32)

    def emit_head_load(h):
        q_sb = ld.tile([P, S // P, D], BF16, tag="q_sb", name=f"q_sb_{h}")
        nc.gpsimd.dma_start(out=q_sb, in_=q_h[h].rearrange("(t p) d -> p t d", p=P))
        k_sb = ld.tile([P, T, D], BF16, tag="k_sb", name=f"k_sb_{h}")
        nc.gpsimd.dma_start(out=k_sb, in_=k_h[h].rearrange("(t p) d -> p t d", p=P))
        v_sb = ld.tile([P, T, D + 1], BF16, tag="v_sb", name=f"v_sb_{h}")
        nc.gpsimd.dma_start(
            out=v_sb, in_=v_h[h].rearrange("(t p) d -> p t d", p=P)
        )
        return q_sb, k_sb, v_sb

    def head_prep_thunks(h):
        # Q^T [64, S] / K^T [64, cap] bf16, each slice duplicated onto
        # partitions 64-127 right after it is built (SBUF->SBUF DMA) so mm1
        # row-tile pairs never wait long on a duplicate.  Split into small
        # thunks so the PE work spreads across many pipeline units.
        q_sb, k_sb, _ = heads[h]

        def alloc():
            qT = qkT.tile([2 * D, S], BF16, tag="qT", name=f"qT_{h}")
            kT = qkT.tile([2 * D, cap], BF16, tag="kT", name=f"kT_{h}")
            headsT[h] = (qT, kT)

        def group(which, t0, nt):
            def f():
                src = q_sb if which == 0 else k_sb
                dst = headsT[h][which]
                cols = slice(t0 * P, (t0 + nt) * P)
                tp = tp_pool.tile(
                    [D, 4 * P], BF16, tag="tp", name=f"tp_{h}_{which}_{t0}"
                )
                for jj in range(nt):
                    nc.tensor.transpose(
                        tp[:, jj * P : (jj + 1) * P], src[:, t0 + jj, :], ident_bf
                    )
                nc.vector.tensor_copy(dst[:D, cols], tp[:, 0 : nt * P])
                nc.sync.dma_start(out=dst[D : 2 * D, cols], in_=dst[:D, cols])

            return f

        def chunks(ntiles):
            return [(t0, min(4, ntiles - t0)) for t0 in range(0, ntiles, 4)]

        kc = chunks(T)
        qc = chunks(S // P)
        first = group(1, *kc[0])
        thunks = [lambda: (alloc(), first())]
        thunks += [group(0, *qc[0])]
        for i in range(1, max(len(kc), len(qc))):  # interleave K and Q groups
            if i < len(kc):
                thunks.append(group(1, *kc[i]))
            if i < len(qc):
                thunks.append(group(0, *qc[i]))
        return thunks

    def emit_epilogue_rest(ep):
        # transpose [65, W] -> W/P tiles of [q=128, 65], normalize by the
        # sums row (column 64 after transposing), store.
        h, q0, accs = ep
        ost = outs_pool.tile([P, W // P, D], F32, tag="ost")
        for j in range(W // P):
            ot = tp_pool.tile([P, D + 1], F32, tag="tp")
            nc.tensor.transpose(
                ot, accs[:, j * P : (j + 1) * P], ident_f32[: D + 1, : D + 1]
            )
            nc.vector.reciprocal(ot[:, D : D + 1], ot[:, D : D + 1])
            nc.vector.tensor_scalar_mul(ost[:, j, :], ot[:, 0:D], ot[:, D : D + 1])
        nc.sync.dma_start(
            out=out_h[h, q0 : q0 + W, :].rearrange("(j p) d -> p j d", p=P),
            in_=ost,
        )

    # Flat pipeline over all (head, window, pair) units.  mm2 lags mm1/exp by
    # MM2_LAG units and epilogues lag one more, so every semaphore wait
    # reaching the in-order PE stream is already satisfied and the matmuls
    # chain back-to-back (drains hidden by the next fill).
    MM2_LAG = 3
    NP = T // 2  # k-tile pairs per window
    units = [(h, w, j) for h in range(H) for w in range(NW) for j in range(NP)]
    heads = {0: emit_head_load(0)}
    headsT = {}
    accs_by_window = {}
    pTs = {}
    pending_epi = []
    work_queue = []
    for t in head_prep_thunks(0):
        t()

    def emit_mm2(i):
        h, w, j = units[i]
        acc = accs_by_window[(h, w)]
        v_sb = heads[h][2]
        pT_prev = pTs.pop(i)
        for c, t in ((0, 2 * j), (1, 2 * j + 1)):
            nc.tensor.matmul(
                acc,
                lhsT=v_sb[:, t, :],
                rhs=pT_prev[:, c * W : (c + 1) * W],
                start=(j == 0 and c == 0),
                stop=(j == NP - 1 and c == 1),
            )
        if j == NP - 1:  # window done: drain accumulator, defer the rest
            accs = accs_pool.tile([D + 1, W], F32, tag="accs")
            nc.vector.tensor_copy(accs, acc)
            del accs_by_window[(h, w)]
            pending_epi.append((i + 1, (h, w * W, accs)))

    for i, (h, w, j) in enumerate(units):
        if w == 0 and j == 0 and h > 1:
            del heads[h - 2], headsT[h - 2]
        qT, kT = headsT[h]
        if j == 0:
            accs_by_window[(h, w)] = acc_pool.tile(
                [D + 1, W], F32, tag="acc", name=f"acc_{h}_{w}"
            )
        q0 = w * W
        # one PSUM tile holds S^T for both k-tiles of the pair side by side,
        # written by two concurrently-executing row-tiled matmuls
        st = st_pool.tile([P, 2 * W], F32, tag="st")
        for c, (t, lo) in enumerate(((2 * j, 0), (2 * j + 1, D))):
            nc.tensor.matmul(
                st[:, c * W : (c + 1) * W],
                lhsT=kT[lo : lo + D, t * P : (t + 1) * P],
                rhs=qT[lo : lo + D, q0 : q0 + W],
                start=True,
                stop=True,
            )
        pT = ppool.tile([P, 2 * W], BF16, tag="pT")
        nc.scalar.activation(
            out=pT, in_=st, func=mybir.ActivationFunctionType.Exp, scale=SCALE
        )
        pTs[i] = pT
        if i >= MM2_LAG:
            emit_mm2(i - MM2_LAG)
        while pending_epi and pending_epi[0][0] <= i - MM2_LAG:
            emit_epilogue_rest(pending_epi.pop(0)[1])
        if j == min(2, NP - 1) and w == 0 and h + 1 < H:
            heads[h + 1] = emit_head_load(h + 1)
        if j == 0 and w == 1 and h + 1 < H:
            work_queue.extend(head_prep_thunks(h + 1))
        if work_queue:
            work_queue.pop(0)()
    for i in range(len(units) - MM2_LAG, len(units)):
        emit_mm2(i)
    for _, ep in pending_epi:
        emit_epilogue_rest(ep)


def build_nc(T):
    nc = bacc.Bacc("TRN2", target_bir_lowering=False, debug=False, num_devices=N_CORES)
    q = nc.declare_dram_parameter("q", [H, S, D], F32, isOutput=False)
    k = nc.declare_dram_parameter("k", [H, T * P, D], F32, isOutput=False)
    v = nc.declare_dram_parameter("v", [H, T * P, D + 1], F32, isOutput=False)
    out = nc.declare_dram_parameter("out", [H, S, D], F32, isOutput=True)
    from contextlib import ExitStack

    with tile.TileContext(nc) as tc, ExitStack() as ctx:
        emit_core_program(ctx, nc, tc, T, q.ap(), k.ap(), v.ap(), out.ap())
    nc.compile()
    return nc


_NC_CACHE = {}


def get_nc(T):
    if T not in _NC_CACHE:
        _NC_CACHE[T] = build_nc(T)
    return _NC_CACHE[T]


def make_in_maps(q, k, v, mask):
    """Pack unmasked keys per batch, shard [B,NH,S,D] inputs across 8 cores."""
    qf = np.asarray(q, dtype=np.float32)
    kf = np.asarray(k, dtype=np.float32)
    vf = np.asarray(v, dtype=np.float32)
    mf = np.asarray(mask, dtype=np.int32).reshape(B, S)

    idxs = [np.flatnonzero(mf[b] == 0) for b in range(B)]
    maxcnt = max(len(ix) for ix in idxs)
    cap = min(S, max(256, -(-maxcnt // 256) * 256))
    T = cap // P

    # per-batch packed K and V' (ones column = valid flag), zero-padded to cap
    kp = np.zeros((B, NH, cap, D), dtype=np.float32)
    vp = np.zeros((B, NH, cap, D + 1), dtype=np.float32)
    for b in range(B):
        n = len(idxs[b])
        kp[b, :, :n, :] = kf[b][:, idxs[b], :]
        vp[b, :, :n, :D] = vf[b][:, idxs[b], :]
        vp[b, :, :n, D] = 1.0

    qf = qf.reshape(B * NH, S, D)
    kp = kp.reshape(B * NH, cap, D)
    vp = vp.reshape(B * NH, cap, D + 1)
    in_maps = []
    for c in range(N_CORES):
        lo = c * H
        in_maps.append(
            {
                "q": np.ascontiguousarray(qf[lo : lo + H]),
                "k": np.ascontiguousarray(kp[lo : lo + H]),
                "v": np.ascontiguousarray(vp[lo : lo + H]),
            }
        )
    return T, in_maps


def kernel(q, k, v, mask):
    from concourse.bass_utils import run_bass_kernel_spmd

    T, in_maps = make_in_maps(q, k, v, mask)
    nc = get_nc(T)
    try:
        res = run_bass_kernel_spmd(nc, in_maps, list(range(N_CORES))).results
    except Exception:
        # the axon execute path occasionally throws a transient INTERNAL
        # error right after a fresh NEFF compile; one retry clears it
        res = run_bass_kernel_spmd(nc, in_maps, list(range(N_CORES))).results
    out = np.concatenate([res[c]["out"] for c in range(N_CORES)], axis=0)
    return out.reshape(B, NH, S, D)


if __name__ == "__main__":
    nc = build_nc(10)
    print("built ok")


# revision 27
# speedup vs baseline: 1.5288x; 1.3185x over previous
"""Trainium2 Bass kernel for nn_BaseAttention (B=4, H=16, S=2048, D=64, key-mask).

Strategy (8 NeuronCores, batch*head sharded, 8 heads per core; each core's 8
heads share one batch's mask):
  Host-side key packing: the key mask is per-(batch, key) and masks ~half the
  keys with -1e4 (whose exp underflows to exactly 0 in f32).  kernel() gathers
  the unmasked keys of K and V per batch, appends a "ones" column to V (for
  the softmax denominator), and zero-pads to a common capacity `cap` (multiple
  of 256).  The device kernel then runs dense attention over cap keys instead
  of 2048 — exactly equivalent math, ~half the exp/matmul work.

  Per head on device (Q: [S,D], Kp: [cap,D], Vp': [cap,D+1], all f32 in HBM):
    - Load with fp32->bf16 cast during DMA (SWDGE).
    - PE-transpose Q,Kp tiles -> Q^T [64,S], Kp^T [64,cap] bf16, duplicated
      onto partitions 64-127 so mm1 can run two k-tiles in the two row halves.
    - Scores transposed: S^T[k, q] = Kp @ Q^T, fp32 PSUM, one [128, 2*512]
      tile per k-tile pair; one ScalarE pass computes P^T = Exp(S^T/8).
      No max-subtraction (scores ~N(0,1)); no additive mask (handled by the
      packing; padded K rows give exp(0)=1 but their V' rows are zero).
    - mm2 accumulates out'^T [65, q] over k; the ones-column row is the
      softmax denominator (zero for padded keys).
    - Reciprocal of sums, PE-transpose [65, q] -> [q, 65], scale, store.
  Emission is a flat software pipeline over (head, window, k-pair) units with
  mm2 and epilogues lagging so the in-order PE stream never stalls.

Self-contained: hardcodes shapes; imports concourse from /opt/trn_rl_repo.
"""

import sys

if "/opt/trn_rl_repo" not in sys.path:
    sys.path.insert(0, "/opt/trn_rl_repo")

import numpy as np

import concourse.bass as bass
import concourse.mybir as mybir
import concourse.tile as tile
from concourse import bacc
from concourse.masks import make_identity

F32 = mybir.dt.float32
BF16 = mybir.dt.bfloat16
I32 = mybir.dt.int32

N_CORES = 8
B, NH, S, D = 4, 16, 2048, 64
H = (B * NH) // N_CORES  # heads per core = 8
P = 128                  # partitions / k-tile size
W = 512                  # q-window width (= fp32 PSUM bank limit per matmul)
NW = S // W              # 4 q-windows per head
SCALE = 1.0 / 8.0        # 1/sqrt(D)


def emit_core_program(ctx, nc, tc, T, q_h, k_h, v_h, out_h):
    """Per-core Tile program. q/out: [H, S, D]; k: [H, T*128, D]; v: [H, T*128, D+1]."""
    cap = T * P
    pool = lambda *a, **kw: ctx.enter_context(tc.tile_pool(*a, **kw))
    singles = pool(name="singles", bufs=1)
    ld = pool(name="ld", bufs=2)            # SBUF head staging (bf16)
    qkT = pool(name="qkT", bufs=2)          # SBUF Q^T/K^T (both row halves)
    ppool = pool(name="p", bufs=5)          # SBUF P^T tiles (lagged mm2)
    accs_pool = pool(name="accs", bufs=2)   # SBUF drained accumulators
    outs_pool = pool(name="outs", bufs=2)   # SBUF output staging
    st_pool = pool(name="st", bufs=2, space="PSUM")    # S^T pair tiles (2 banks ea)
    acc_pool = pool(name="acc", bufs=2, space="PSUM")  # out'^T accum (1 bank ea)
    tp_pool = pool(name="tp", bufs=2, space="PSUM")    # transposes (1 bank ea)

    ident_bf = singles.tile([P, P], BF16)
    make_identity(nc, ident_bf)
    ident_f32 = singles.tile([P, P], F32)
    make_identity(nc, ident_f32)

    def emit_head_load(h):
        # Q^T/K^T arrive pre-transposed from the host, already duplicated
        # into both partition halves for mm1 row tiling — no device
        # transposes at all.
        qT = ld.tile([2 * D, S], BF16, tag="qT", name=f"qT_{h}")
        nc.gpsimd.dma_start(out=qT, in_=q_h[h])
        kT = ld.tile([2 * D, cap], BF16, tag="kT", name=f"kT_{h}")
        nc.gpsimd.dma_start(out=kT, in_=k_h[h])
        v_sb = ld.tile([P, T, D + 1], BF16, tag="v_sb", name=f"v_sb_{h}")
        nc.gpsimd.dma_start(
            out=v_sb, in_=v_h[h].rearrange("(t p) d -> p t d", p=P)
        )
        headsT[h] = (qT, kT)
        return qT, kT, v_sb

    def emit_epilogue_rest(ep):
        # transpose [65, W] -> W/P tiles of [q=128, 65], normalize by the
        # sums row (column 64 after transposing), store.
        h, q0, accs = ep
        ost = outs_pool.tile([P, W // P, D], F32, tag="ost")
        for j in range(W // P):
            ot = tp_pool.tile([P, D + 1], F32, tag="tp")
            nc.tensor.transpose(
                ot, accs[:, j * P : (j + 1) * P], ident_f32[: D + 1, : D + 1]
            )
            nc.vector.reciprocal(ot[:, D : D + 1], ot[:, D : D + 1])
            nc.vector.tensor_scalar_mul(ost[:, j, :], ot[:, 0:D], ot[:, D : D + 1])
        nc.sync.dma_start(
            out=out_h[h, q0 : q0 + W, :].rearrange("(j p) d -> p j d", p=P),
            in_=ost,
        )

    # Flat pipeline over all (head, window, pair) units.  mm2 lags mm1/exp by
    # MM2_LAG units and epilogues lag one more, so every semaphore wait
    # reaching the in-order PE stream is already satisfied and the matmuls
    # chain back-to-back (drains hidden by the next fill).
    MM2_LAG = 3
    NP = T // 2  # k-tile pairs per window
    units = [(h, w, j) for h in range(H) for w in range(NW) for j in range(NP)]
    headsT = {}
    heads = {0: emit_head_load(0)}
    accs_by_window = {}
    pTs = {}
    pending_epi = []

    def emit_mm2(i):
        h, w, j = units[i]
        acc = accs_by_window[(h, w)]
        v_sb = heads[h][2]
        pT_prev = pTs.pop(i)
        for c, t in ((0, 2 * j), (1, 2 * j + 1)):
            nc.tensor.matmul(
                acc,
                lhsT=v_sb[:, t, :],
                rhs=pT_prev[:, c * W : (c + 1) * W],
                start=(j == 0 and c == 0),
                stop=(j == NP - 1 and c == 1),
            )
        if j == NP - 1:  # window done: drain accumulator, defer the rest
            accs = accs_pool.tile([D + 1, W], F32, tag="accs")
            nc.vector.tensor_copy(accs, acc)
            del accs_by_window[(h, w)]
            pending_epi.append((i + 1, (h, w * W, accs)))

    for i, (h, w, j) in enumerate(units):
        if w == 0 and j == 0 and h > 1:
            del heads[h - 2], headsT[h - 2]
        qT, kT = headsT[h]
        if j == 0:
            accs_by_window[(h, w)] = acc_pool.tile(
                [D + 1, W], F32, tag="acc", name=f"acc_{h}_{w}"
            )
        q0 = w * W
        # one PSUM tile holds S^T for both k-tiles of the pair side by side,
        # written by two concurrently-executing row-tiled matmuls
        st = st_pool.tile([P, 2 * W], F32, tag="st")
        for c, (t, lo) in enumerate(((2 * j, 0), (2 * j + 1, D))):
            nc.tensor.matmul(
                st[:, c * W : (c + 1) * W],
                lhsT=kT[lo : lo + D, t * P : (t + 1) * P],
                rhs=qT[lo : lo + D, q0 : q0 + W],
                start=True,
                stop=True,
            )
        pT = ppool.tile([P, 2 * W], BF16, tag="pT")
        nc.scalar.activation(
            out=pT, in_=st, func=mybir.ActivationFunctionType.Exp, scale=SCALE
        )
        pTs[i] = pT
        if i >= MM2_LAG:
            emit_mm2(i - MM2_LAG)
        while pending_epi and pending_epi[0][0] <= i - MM2_LAG:
            emit_epilogue_rest(pending_epi.pop(0)[1])
        if j == min(2, NP - 1) and w == 0 and h + 1 < H:
            heads[h + 1] = emit_head_load(h + 1)
    for i in range(len(units) - MM2_LAG, len(units)):
        emit_mm2(i)
    for _, ep in pending_epi:
        emit_epilogue_rest(ep)


def build_nc(T):
    nc = bacc.Bacc("TRN2", target_bir_lowering=False, debug=False, num_devices=N_CORES)
    q = nc.declare_dram_parameter("q", [H, 2 * D, S], F32, isOutput=False)
    k = nc.declare_dram_parameter("k", [H, 2 * D, T * P], F32, isOutput=False)
    v = nc.declare_dram_parameter("v", [H, T * P, D + 1], F32, isOutput=False)
    out = nc.declare_dram_parameter("out", [H, S, D], F32, isOutput=True)
    from contextlib import ExitStack

    with tile.TileContext(nc) as tc, ExitStack() as ctx:
        emit_core_program(ctx, nc, tc, T, q.ap(), k.ap(), v.ap(), out.ap())
    nc.compile()
    return nc


_NC_CACHE = {}


def get_nc(T):
    if T not in _NC_CACHE:
        _NC_CACHE[T] = build_nc(T)
    return _NC_CACHE[T]


def make_in_maps(q, k, v, mask):
    """Pack unmasked keys per batch, shard [B,NH,S,D] inputs across 8 cores."""
    qf = np.asarray(q, dtype=np.float32)
    kf = np.asarray(k, dtype=np.float32)
    vf = np.asarray(v, dtype=np.float32)
    mf = np.asarray(mask, dtype=np.int32).reshape(B, S)

    idxs = [np.flatnonzero(mf[b] == 0) for b in range(B)]
    maxcnt = max(len(ix) for ix in idxs)
    cap = min(S, max(256, -(-maxcnt // 256) * 256))
    T = cap // P

    # per-batch packed K and V' (ones column = valid flag), zero-padded to cap
    kp = np.zeros((B, NH, cap, D), dtype=np.float32)
    vp = np.zeros((B, NH, cap, D + 1), dtype=np.float32)
    for b in range(B):
        n = len(idxs[b])
        kp[b, :, :n, :] = kf[b][:, idxs[b], :]
        vp[b, :, :n, :D] = vf[b][:, idxs[b], :]
        vp[b, :, :n, D] = 1.0

    qT = qf.reshape(B * NH, S, D).transpose(0, 2, 1)
    qTd = np.concatenate([qT, qT], axis=1)  # [BNH, 2D, S]
    kT = kp.reshape(B * NH, cap, D).transpose(0, 2, 1)
    kTd = np.concatenate([kT, kT], axis=1)  # [BNH, 2D, cap]
    vp = vp.reshape(B * NH, cap, D + 1)
    in_maps = []
    for c in range(N_CORES):
        lo = c * H
        in_maps.append(
            {
                "q": np.ascontiguousarray(qTd[lo : lo + H]),
                "k": np.ascontiguousarray(kTd[lo : lo + H]),
                "v": np.ascontiguousarray(vp[lo : lo + H]),
            }
        )
    return T, in_maps


def kernel(q, k, v, mask):
    from concourse.bass_utils import run_bass_kernel_spmd

    T, in_maps = make_in_maps(q, k, v, mask)
    nc = get_nc(T)
    try:
        res = run_bass_kernel_spmd(nc, in_maps, list(range(N_CORES))).results
    except Exception:
        # the axon execute path occasionally throws a transient INTERNAL
        # error right after a fresh NEFF compile; one retry clears it
        res = run_bass_kernel_spmd(nc, in_maps, list(range(N_CORES))).results
    out = np.concatenate([res[c]["out"] for c in range(N_CORES)], axis=0)
    return out.reshape(B, NH, S, D)


if __name__ == "__main__":
    nc = build_nc(10)
    print("built ok")
